# revision 1
# baseline (speedup 1.0000x reference)
"""Triangle attention (starting node) Bass kernel for 8 trn2 NeuronCores.

Math (B=1, N=256, D=128, H=4, E=32):
  bias[h,j,k] = sum_d P[j,k,d] Wb[d,h]
  q[h,i,j,e]  = sum_d P[i,j,d] Wq[d,h*E+e]   (k,v analogous)
  S[i,h,j,k]  = (q . k) * E**-0.5 + bias[h,j,k]
  out[i,j,:]  = (softmax_k S @ v) merged over h, @ Wo

Sharding: rows i are split across 8 cores (32 rows each). The bias couples all
rows, so pass 1 computes each core's 32 bias columns and the host concatenates
the shards (layout-only work); pass 2 runs attention per row shard.

On-chip layout is "T-form": scores are built transposed, ST[k, j] per head, so
softmax normalization sums over the partition axis (done on the PE with a ones
matmul, replicated x32 for free) and the AV matmul consumes ST directly with
no transpose of the attention matrix. The host supplies pairwise_repr already
transposed to [d, i*N+token] so every on-chip matmul operand has its
contraction axis on partitions.
"""

import os
from contextlib import ExitStack

import numpy as np

N = 256
D = 128
H = 4
E = 32
NCORES = 8
RPC = N // NCORES  # rows per core
SCALE = float(E) ** -0.5
F32 = None  # set lazily to mybir.dt.float32

_cache = {}


def _dt():
    import concourse.mybir as mybir

    return mybir.dt.float32


_legal_ctr = [0]


def _legalize_waits(nc):
    """Walrus caps semaphore wait-commands per lowered instruction (LDWEIGHTS
    holds only one). Hoist excess waits of every non-Drain instruction into
    fresh single-wait NoOps on the same engine, inserted right before it —
    same wait point, so timing/deadlock semantics are unchanged."""
    import bass_rust

    for fn in nc.m.functions:
        for blk in fn.blocks:
            ins = blk.instructions
            i = 0
            while i < len(ins):
                inst = ins[i]
                si = inst.sync_info
                if si is None or inst.engine is None:
                    i += 1
                    continue
                waits = si.on_wait
                if len(waits) <= 1:
                    i += 1
                    continue
                for w in waits[:-1]:
                    _legal_ctr[0] += 1
                    n = bass_rust.InstNoOp(name=f"I-lgl-{_legal_ctr[0]}")
                    n.engine = inst.engine
                    n.sync_info = bass_rust.SyncInfo(on_wait=[w], on_update=[])
                    ins.insert(i, n)
                    i += 1
                si.on_wait = [waits[-1]]
                inst.sync_info = si
                i += 1


def _build_pass1():
    """Per-core bias shard: bshard[kh, k, h*RPC + jl] = sum_d xT[d, jl*N+kh*128+k] * Wb[d, h]."""
    import concourse.bass as bass
    import concourse.mybir as mybir
    import concourse.tile as tile

    f32 = _dt()
    nc = bass.Bass("TRN2", target_bir_lowering=False, debug=False,
                   enable_asserts=False, num_devices=NCORES)
    xT = nc.dram_tensor("xT", [D, RPC * N], f32, kind="ExternalInput").ap()
    wb = nc.dram_tensor("wb", [D, H], f32, kind="ExternalInput").ap()
    bshard = nc.dram_tensor("bshard", [2, 128, H * RPC], f32, kind="ExternalOutput").ap()

    with ExitStack() as ctx:
        tc = ctx.enter_context(tile.TileContext(nc))
        singles = ctx.enter_context(tc.tile_pool(name="singles", bufs=1))
        ps = ctx.enter_context(tc.tile_pool(name="ps", bufs=2, space="PSUM"))

        wb_sb = singles.tile([D, H], f32)
        nc.sync.dma_start(out=wb_sb, in_=wb)
        xt_sb = singles.tile([D, RPC * N], f32)
        for c in range(4):
            sl = slice(c * RPC * N // 4, (c + 1) * RPC * N // 4)
            nc.sync.dma_start(out=xt_sb[:, sl], in_=xT[:, sl])

        st = singles.tile([128, 2 * H * RPC], f32)
        for kh in range(2):
            pb = ps.tile([128, RPC * H], f32)  # [k, jl*H + h]
            for jl in range(RPC):
                nc.tensor.matmul(
                    pb[:, jl * H:(jl + 1) * H],
                    xt_sb[:, jl * N + kh * 128: jl * N + kh * 128 + 128],
                    wb_sb,
                    start=True, stop=True,
                )
            # st[:, kh*128 + h*RPC + jl] = pb[:, jl*H + h]
            nc.vector.tensor_copy(
                st[:, kh * H * RPC:(kh + 1) * H * RPC].rearrange(
                    "p (h j) -> p h j", h=H),
                pb.rearrange("p (j h) -> p h j", h=H),
            )
        for kh in range(2):
            nc.sync.dma_start(out=bshard[kh],
                              in_=st[:, kh * H * RPC:(kh + 1) * H * RPC])
    return nc


def _build_pass2():
    import concourse.bass as bass
    import concourse.mybir as mybir
    import concourse.tile as tile
    from concourse.masks import make_identity

    f32 = _dt()
    AF = mybir.ActivationFunctionType
    nc = bass.Bass("TRN2", target_bir_lowering=False, debug=False,
                   enable_asserts=False, num_devices=NCORES)
    xT = nc.dram_tensor("xT", [D, RPC * N], f32, kind="ExternalInput").ap()
    biasT = nc.dram_tensor("biasT", [2, 128, H * N], f32, kind="ExternalInput").ap()
    wq = nc.dram_tensor("wq", [D, D], f32, kind="ExternalInput").ap()
    wk = nc.dram_tensor("wk", [D, D], f32, kind="ExternalInput").ap()
    wv = nc.dram_tensor("wv", [D, D], f32, kind="ExternalInput").ap()
    wo = nc.dram_tensor("wo", [D, D], f32, kind="ExternalInput").ap()
    outT = nc.dram_tensor("outT", [RPC, D, N], f32, kind="ExternalOutput").ap()

    RB = 8  # rows per projection batch
    with ExitStack() as ctx:
        tc = ctx.enter_context(tile.TileContext(nc))
        singles = ctx.enter_context(tc.tile_pool(name="singles", bufs=1))
        qk_pool = ctx.enter_context(tc.tile_pool(name="qk", bufs=4))
        v_pool = ctx.enter_context(tc.tile_pool(name="v", bufs=3))
        es_pool = ctx.enter_context(tc.tile_pool(name="es", bufs=4))
        sm_pool = ctx.enter_context(tc.tile_pool(name="sm", bufs=3))
        out_pool = ctx.enter_context(tc.tile_pool(name="outp", bufs=3))
        s_psum = ctx.enter_context(tc.tile_pool(name="spsum", bufs=2, space="PSUM"))
        o_psum = ctx.enter_context(tc.tile_pool(name="opsum", bufs=2, space="PSUM"))
        m_psum = ctx.enter_context(tc.tile_pool(name="mpsum", bufs=2, space="PSUM"))

        wq_sb = singles.tile([D, D], f32)
        wk_sb = singles.tile([D, D], f32)
        wv_sb = singles.tile([D, D], f32)
        wo_sb = singles.tile([D, D], f32)
        ident = singles.tile([128, 128], f32)
        ones = singles.tile([128, E], f32)
        bias_sb = singles.tile([128, 2 * H * N], f32)  # [k, kh*1024 + h*256 + j]
        xt_sb = singles.tile([D, RPC * N], f32)

        nc.sync.dma_start(out=wq_sb, in_=wq)
        nc.sync.dma_start(out=wk_sb, in_=wk)
        nc.sync.dma_start(out=wv_sb, in_=wv)
        nc.sync.dma_start(out=wo_sb, in_=wo)
        make_identity(nc, ident)
        nc.vector.memset(ones, 1.0)
        for kh in range(2):
            nc.sync.dma_start(out=bias_sb[:, kh * H * N:(kh + 1) * H * N],
                              in_=biasT[kh])
        for c in range(8):
            sl = slice(c * RPC * N // 8, (c + 1) * RPC * N // 8)
            nc.sync.dma_start(out=xt_sb[:, sl], in_=xT[:, sl])

        for rb in range(RPC // RB):
            # --- projections for RB rows: qT/kT [he, rb-local row * N + token]
            qT = qk_pool.tile([128, RB * N], f32, tag="qT")
            kT = qk_pool.tile([128, RB * N], f32, tag="kT")
            for m, (wsb, dst, scl) in enumerate(
                    [(wq_sb, qT, SCALE), (wk_sb, kT, 1.0)]):
                for c in range(RB * N // 512):
                    pp = m_psum.tile([128, 512], f32, tag="m")
                    nc.tensor.matmul(
                        pp,
                        wsb,
                        xt_sb[:, rb * RB * N + c * 512: rb * RB * N + (c + 1) * 512],
                        start=True, stop=True)
                    if scl == 1.0:
                        nc.vector.tensor_copy(dst[:, c * 512:(c + 1) * 512], pp)
                    else:
                        nc.vector.tensor_scalar_mul(
                            dst[:, c * 512:(c + 1) * 512], pp, scl)

            for rl in range(RB):
                r = rb * RB + rl
                roff = rb * RB * N + rl * N
                # --- v for this row: v_sb[ktok, half*128 + he]
                v_sb = v_pool.tile([128, N], f32, tag="v")
                pv = m_psum.tile([128, 512], f32, tag="m")
                for half in range(2):
                    nc.tensor.matmul(
                        pv[:, half * 128:(half + 1) * 128],
                        xt_sb[:, roff + half * 128: roff + half * 128 + 128],
                        wv_sb,
                        start=True, stop=True)
                nc.vector.tensor_copy(v_sb, pv[:, 0:N])

                # --- scores + exp, per k-half chunk [128, H*N]
                est = []
                for kh in range(2):
                    sp = s_psum.tile([128, H * N], f32, tag="s")
                    for h in range(H):
                        nc.tensor.matmul(
                            sp[:, h * N:(h + 1) * N],
                            ident,
                            bias_sb[:, kh * H * N + h * N: kh * H * N + (h + 1) * N],
                            start=True, stop=False)
                        nc.tensor.matmul(
                            sp[:, h * N:(h + 1) * N],
                            kT[32 * h:32 * h + 32, rl * N + kh * 128: rl * N + kh * 128 + 128],
                            qT[32 * h:32 * h + 32, rl * N: (rl + 1) * N],
                            start=False, stop=True,
                            tile_position=(32 * h, 0))
                    es = es_pool.tile([128, H * N], f32, tag="es")
                    nc.scalar.activation(es, sp, AF.Exp)
                    est.append(es)

                # --- rowsums (replicated x32 via ones[128,E]) and AV
                po = o_psum.tile([128, 512], f32, tag="o")
                for h in range(H):
                    for kh in range(2):
                        nc.tensor.matmul(
                            po[32 * h:32 * h + 32, 256:512],
                            ones,
                            est[kh][:, h * N:(h + 1) * N],
                            start=(kh == 0), stop=(kh == 1),
                            tile_position=(0, 32 * h))
                for h in range(H):
                    for kh in range(2):
                        nc.tensor.matmul(
                            po[32 * h:32 * h + 32, 0:256],
                            v_sb[:, kh * 128 + 32 * h: kh * 128 + 32 * h + 32],
                            est[kh][:, h * N:(h + 1) * N],
                            start=(kh == 0), stop=(kh == 1),
                            tile_position=(0, 32 * h))

                rs_rec = sm_pool.tile([128, N], f32, tag="rs")
                nc.vector.reciprocal(rs_rec, po[:, 256:512])
                oT_sb = sm_pool.tile([128, N], f32, tag="oT")
                nc.vector.tensor_mul(oT_sb, po[:, 0:256], rs_rec)

                # --- output projection: outT[d, j] = sum_he Wo[he,d] oT[he,j]
                pf = m_psum.tile([128, 512], f32, tag="m")
                nc.tensor.matmul(pf[:, 0:N], wo_sb, oT_sb, start=True, stop=True)
                o_sb = out_pool.tile([128, N], f32, tag="osb")
                nc.vector.tensor_copy(o_sb, pf[:, 0:N])
                nc.sync.dma_start(out=outT[r], in_=o_sb)
    return nc


def _get_programs():
    if "nc1" not in _cache:
        _cache["nc1"] = _build_pass1()
        _cache["nc2"] = _build_pass2()
        _legalize_waits(_cache["nc1"])
        _legalize_waits(_cache["nc2"])
    return _cache["nc1"], _cache["nc2"]


def kernel(pairwise_repr, mask, Wb, Wq, Wk, Wv, Wo):
    from concourse.bass_utils import run_bass_kernel_spmd

    nc1, nc2 = _get_programs()

    x = np.ascontiguousarray(np.asarray(pairwise_repr, dtype=np.float32)[0])
    # xT[d, i*N + t] = x[i, t, d]
    xT = np.ascontiguousarray(x.reshape(N * N, D).T)
    shards = [np.ascontiguousarray(xT[:, c * RPC * N:(c + 1) * RPC * N])
              for c in range(NCORES)]
    wb = np.ascontiguousarray(np.asarray(Wb, np.float32))
    wq = np.ascontiguousarray(np.asarray(Wq, np.float32))
    wk = np.ascontiguousarray(np.asarray(Wk, np.float32))
    wv = np.ascontiguousarray(np.asarray(Wv, np.float32))
    wo = np.ascontiguousarray(np.asarray(Wo, np.float32))

    trace = False  # NTFF tracing unavailable under this axon build
    core_ids = list(range(NCORES))

    in1 = [{"xT": shards[c], "wb": wb} for c in range(NCORES)]
    kernel._last_in1 = in1
    res1 = run_bass_kernel_spmd(nc1, in1, core_ids=core_ids, trace=trace)
    # bshard [2, 128, H*RPC] -> [2, 128, H, RPC]; concat over cores on j
    bias_full = np.concatenate(
        [res1.results[c]["bshard"].reshape(2, 128, H, RPC)
         for c in range(NCORES)], axis=3)
    biasT = np.ascontiguousarray(bias_full.reshape(2, 128, H * N))

    in2 = [{"xT": shards[c], "biasT": biasT, "wq": wq, "wk": wk,
            "wv": wv, "wo": wo} for c in range(NCORES)]
    kernel._last_in2 = in2
    res2 = run_bass_kernel_spmd(nc2, in2, core_ids=core_ids, trace=trace)

    kernel._last = (res1, res2)
    # outT [RPC, D, N] per core -> out[0, 32c+r, j, d] = outT_c[r, d, j]
    o = np.stack([res2.results[c]["outT"] for c in range(NCORES)])
    out = o.transpose(0, 1, 3, 2).reshape(1, N, N, D)
    return np.ascontiguousarray(out.astype(np.float32))



# revision 8
# speedup vs baseline: 80.8313x; 80.8313x over previous
"""Triangle attention (starting node) Bass kernel for 8 trn2 NeuronCores.

Math (B=1, N=256, D=128, H=4, E=32):
  bias[h,j,k] = sum_d P[j,k,d] Wb[d,h]
  q[h,i,j,e]  = sum_d P[i,j,d] Wq[d,h*E+e]   (k,v analogous)
  S[i,h,j,k]  = (q . k) * E**-0.5 + bias[h,j,k]
  out[i,j,:]  = (softmax_k S @ v) merged over h, @ Wo

Sharding: rows i are split across 8 cores (32 rows each). The bias couples all
rows, so each core receives the FULL pairwise tensor (replicated input, in
transposed [d, j*N+k] form) and recomputes the full bias on-chip — one kernel
launch, no host round trip between a bias pass and an attention pass.

On-chip layout is "T-form": scores are built transposed, ST[k, j] per head, so
softmax normalization sums over the partition axis (done on the PE with a ones
matmul, replicated x32 for free) and the AV matmul consumes ST directly with
no transpose of the attention matrix. The bias add initializes each score
PSUM group via an identity matmul (start=True) — a PSUM accumulation group
whose first matmul sits at a non-zero tile_position faults at runtime on this
toolchain, so the ident matmul must come first. The 1/sqrt(E) scale is folded
into Wq once at load time.
"""

import os
from contextlib import ExitStack

import numpy as np

N = 256
D = 128
H = 4
E = 32
NCORES = 8
RPC = N // NCORES  # rows per core
SCALE = float(E) ** -0.5

_cache = {}


def _dt():
    import concourse.mybir as mybir

    return mybir.dt.float32


_legal_ctr = [0]


def _legalize_waits(nc):
    """Walrus caps semaphore wait-commands per lowered instruction (LDWEIGHTS
    holds only one). Hoist excess waits of every non-Drain instruction into
    fresh single-wait NoOps on the same engine, inserted right before it —
    same wait point, so timing/deadlock semantics are unchanged."""
    import bass_rust

    for fn in nc.m.functions:
        for blk in fn.blocks:
            ins = blk.instructions
            i = 0
            while i < len(ins):
                inst = ins[i]
                si = inst.sync_info
                if si is None or inst.engine is None:
                    i += 1
                    continue
                waits = si.on_wait
                if len(waits) <= 1:
                    i += 1
                    continue
                for w in waits[:-1]:
                    _legal_ctr[0] += 1
                    n = bass_rust.InstNoOp(name=f"I-lgl-{_legal_ctr[0]}")
                    n.engine = inst.engine
                    n.sync_info = bass_rust.SyncInfo(on_wait=[w], on_update=[])
                    ins.insert(i, n)
                    i += 1
                si.on_wait = [waits[-1]]
                inst.sync_info = si
                i += 1


def _build():
    import concourse.bass as bass
    import concourse.mybir as mybir
    import concourse.tile as tile
    from concourse.masks import make_identity

    f32 = _dt()
    AF = mybir.ActivationFunctionType
    nc = bass.Bass("TRN2", target_bir_lowering=False, debug=False,
                   enable_asserts=False, num_devices=NCORES)
    xT = nc.dram_tensor("xT", [D, RPC * N], f32, kind="ExternalInput").ap()
    xF = nc.dram_tensor("xF", [D, N * N], f32, kind="ExternalInput").ap()
    wb = nc.dram_tensor("wb", [D, H], f32, kind="ExternalInput").ap()
    wq = nc.dram_tensor("wq", [D, D], f32, kind="ExternalInput").ap()
    wk = nc.dram_tensor("wk", [D, D], f32, kind="ExternalInput").ap()
    wv = nc.dram_tensor("wv", [D, D], f32, kind="ExternalInput").ap()
    wo = nc.dram_tensor("wo", [D, D], f32, kind="ExternalInput").ap()
    outT = nc.dram_tensor("outT", [RPC, D, N], f32, kind="ExternalOutput").ap()

    RB = 8       # rows per projection batch
    XCH = 4096   # xF streaming chunk (columns); 16 j-blocks per chunk
    with ExitStack() as ctx:
        tc = ctx.enter_context(tile.TileContext(nc))
        singles = ctx.enter_context(tc.tile_pool(name="singles", bufs=1))
        xf_pool = ctx.enter_context(tc.tile_pool(name="xf", bufs=2))
        qk_pool = ctx.enter_context(tc.tile_pool(name="qk", bufs=4))
        v_pool = ctx.enter_context(tc.tile_pool(name="v", bufs=3))
        es_pool = ctx.enter_context(tc.tile_pool(name="es", bufs=4))
        sm_pool = ctx.enter_context(tc.tile_pool(name="sm", bufs=3))
        out_pool = ctx.enter_context(tc.tile_pool(name="outp", bufs=3))
        s_psum = ctx.enter_context(tc.tile_pool(name="spsum", bufs=2, space="PSUM"))
        o_psum = ctx.enter_context(tc.tile_pool(name="opsum", bufs=2, space="PSUM"))
        m_psum = ctx.enter_context(tc.tile_pool(name="mpsum", bufs=2, space="PSUM"))

        wb_sb = singles.tile([D, H], f32)
        wq_sb = singles.tile([D, D], f32)
        wk_sb = singles.tile([D, D], f32)
        wv_sb = singles.tile([D, D], f32)
        wo_sb = singles.tile([D, D], f32)
        ones = singles.tile([128, E], f32)
        ident = singles.tile([128, 128], f32)
        bias_sb = singles.tile([128, 2 * H * N], f32)  # [k, kh*1024 + h*256 + j]
        xt_sb = singles.tile([D, RPC * N], f32)

        nc.sync.dma_start(out=wb_sb, in_=wb)
        nc.sync.dma_start(out=wq_sb, in_=wq)
        nc.sync.dma_start(out=wk_sb, in_=wk)
        nc.sync.dma_start(out=wv_sb, in_=wv)
        nc.sync.dma_start(out=wo_sb, in_=wo)
        nc.vector.memset(ones, 1.0)
        make_identity(nc, ident)
        nc.vector.tensor_scalar_mul(wq_sb, wq_sb, SCALE)
        for c in range(8):
            sl = slice(c * RPC * N // 8, (c + 1) * RPC * N // 8)
            nc.sync.dma_start(out=xt_sb[:, sl], in_=xT[:, sl])

        # --- full bias from the replicated pairwise tensor.
        # bias_sb[k, kh*H*N + h*N + j] = sum_d xF[d, j*N + kh*128 + k] wb[d,h]
        bias_4d = bias_sb.rearrange("p (kh h j) -> p kh h j", kh=2, h=H)
        for c in range(N * N // XCH):
            xc = xf_pool.tile([128, XCH], f32, tag="xf")
            nc.sync.dma_start(out=xc, in_=xF[:, c * XCH:(c + 1) * XCH])
            JB = XCH // N  # j-blocks in this chunk
            pb = m_psum.tile([128, JB * 2 * H], f32, tag="m")  # [k, (jl kh h)]
            for jl in range(JB):
                for kh in range(2):
                    nc.tensor.matmul(
                        pb[:, (jl * 2 + kh) * H:(jl * 2 + kh + 1) * H],
                        xc[:, jl * N + kh * 128: jl * N + kh * 128 + 128],
                        wb_sb,
                        start=True, stop=True,
                    )
            nc.vector.tensor_copy(
                bias_4d[:, :, :, c * JB:(c + 1) * JB],
                pb.rearrange("p (j kh h) -> p kh h j", j=JB, kh=2),
            )

        for rb in range(RPC // RB):
            # --- projections for RB rows: qT/kT [he, rb-local row * N + token]
            qT = qk_pool.tile([128, RB * N], f32, tag="qT")
            kT = qk_pool.tile([128, RB * N], f32, tag="kT")
            for wsb, dst in ((wq_sb, qT), (wk_sb, kT)):
                for c in range(RB * N // 512):
                    pp = m_psum.tile([128, 512], f32, tag="m")
                    nc.tensor.matmul(
                        pp,
                        wsb,
                        xt_sb[:, rb * RB * N + c * 512: rb * RB * N + (c + 1) * 512],
                        start=True, stop=True)
                    nc.vector.tensor_copy(dst[:, c * 512:(c + 1) * 512], pp)

            for rl in range(RB):
                r = rb * RB + rl
                roff = rb * RB * N + rl * N
                # --- v for this row: v_sb[ktok, half*128 + he]
                v_sb = v_pool.tile([128, N], f32, tag="v")
                pv = m_psum.tile([128, 512], f32, tag="m")
                for half in range(2):
                    nc.tensor.matmul(
                        pv[:, half * 128:(half + 1) * 128],
                        xt_sb[:, roff + half * 128: roff + half * 128 + 128],
                        wv_sb,
                        start=True, stop=True)
                nc.vector.tensor_copy(v_sb, pv[:, 0:N])

                # --- scores + exp(qk)*exp(bias), per k-half chunk [128, H*N]
                est = []
                for kh in range(2):
                    sp = s_psum.tile([128, H * N], f32, tag="s")
                    for h in range(H):
                        nc.tensor.matmul(
                            sp[:, h * N:(h + 1) * N],
                            ident,
                            bias_sb[:, kh * H * N + h * N: kh * H * N + (h + 1) * N],
                            start=True, stop=False)
                        nc.tensor.matmul(
                            sp[:, h * N:(h + 1) * N],
                            kT[32 * h:32 * h + 32, rl * N + kh * 128: rl * N + kh * 128 + 128],
                            qT[32 * h:32 * h + 32, rl * N: (rl + 1) * N],
                            start=False, stop=True,
                            tile_position=(32 * h, 0))
                    es = es_pool.tile([128, H * N], f32, tag="es")
                    nc.scalar.activation(es, sp, AF.Exp)
                    est.append(es)

                # --- rowsums (replicated x32 via ones[128,E]) and AV
                po = o_psum.tile([128, 512], f32, tag="o")
                for h in range(H):
                    for kh in range(2):
                        nc.tensor.matmul(
                            po[32 * h:32 * h + 32, 256:512],
                            ones,
                            est[kh][:, h * N:(h + 1) * N],
                            start=(kh == 0), stop=(kh == 1),
                            tile_position=(0, 32 * h))
                for h in range(H):
                    for kh in range(2):
                        nc.tensor.matmul(
                            po[32 * h:32 * h + 32, 0:256],
                            v_sb[:, kh * 128 + 32 * h: kh * 128 + 32 * h + 32],
                            est[kh][:, h * N:(h + 1) * N],
                            start=(kh == 0), stop=(kh == 1),
                            tile_position=(0, 32 * h))

                rs_rec = sm_pool.tile([128, N], f32, tag="rs")
                nc.vector.reciprocal(rs_rec, po[:, 256:512])
                oT_sb = sm_pool.tile([128, N], f32, tag="oT")
                nc.vector.tensor_mul(oT_sb, po[:, 0:256], rs_rec)

                # --- output projection: outT[d, j] = sum_he Wo[he,d] oT[he,j]
                pf = m_psum.tile([128, 512], f32, tag="m")
                nc.tensor.matmul(pf[:, 0:N], wo_sb, oT_sb, start=True, stop=True)
                o_sb = out_pool.tile([128, N], f32, tag="osb")
                nc.vector.tensor_copy(o_sb, pf[:, 0:N])
                nc.sync.dma_start(out=outT[r], in_=o_sb)
    return nc


def _get_program():
    if "nc" not in _cache:
        _cache["nc"] = _build()
        _legalize_waits(_cache["nc"])
    return _cache["nc"]


def kernel(pairwise_repr, mask, Wb, Wq, Wk, Wv, Wo):
    from concourse.bass_utils import run_bass_kernel_spmd

    nc = _get_program()

    x = np.ascontiguousarray(np.asarray(pairwise_repr, dtype=np.float32)[0])
    # xT[d, i*N + t] = x[i, t, d]
    xT = np.ascontiguousarray(x.reshape(N * N, D).T)
    shards = [np.ascontiguousarray(xT[:, c * RPC * N:(c + 1) * RPC * N])
              for c in range(NCORES)]
    wb = np.ascontiguousarray(np.asarray(Wb, np.float32))
    wq = np.ascontiguousarray(np.asarray(Wq, np.float32))
    wk = np.ascontiguousarray(np.asarray(Wk, np.float32))
    wv = np.ascontiguousarray(np.asarray(Wv, np.float32))
    wo = np.ascontiguousarray(np.asarray(Wo, np.float32))

    core_ids = list(range(NCORES))
    in_maps = [{"xT": shards[c], "xF": xT, "wb": wb, "wq": wq, "wk": wk,
                "wv": wv, "wo": wo} for c in range(NCORES)]
    kernel._last_in = in_maps
    res = run_bass_kernel_spmd(nc, in_maps, core_ids=core_ids, trace=False)

    kernel._last = res
    # outT [RPC, D, N] per core -> out[0, 32c+r, j, d] = outT_c[r, d, j]
    o = np.stack([res.results[c]["outT"] for c in range(NCORES)])
    out = o.transpose(0, 1, 3, 2).reshape(1, N, N, D)
    return np.ascontiguousarray(out.astype(np.float32))


# revision 10
# speedup vs baseline: 83.7867x; 1.0366x over previous
"""Triangle attention (starting node) Bass kernel for 8 trn2 NeuronCores.

Math (B=1, N=256, D=128, H=4, E=32):
  bias[h,j,k] = sum_d P[j,k,d] Wb[d,h]
  q[h,i,j,e]  = sum_d P[i,j,d] Wq[d,h*E+e]   (k,v analogous)
  S[i,h,j,k]  = (q . k) * E**-0.5 + bias[h,j,k]
  out[i,j,:]  = (softmax_k S @ v) merged over h, @ Wo

Sharding: rows i are split across 8 cores (32 rows each). The bias couples all
rows, so each core receives the FULL pairwise tensor (replicated input, in
transposed [d, j*N+k] form, bf16 to halve the stream) and recomputes the full
bias on-chip — one kernel launch, no host round trip between a bias pass and
an attention pass.

On-chip layout is "T-form": scores are built transposed, ST[k, j] per head, so
softmax normalization sums over the partition axis (done on the PE together
with the AV matmul: the stationary operand is [v_h | ones], so one pass yields
both A@V and the softmax denominator replicated x32) and the AV matmul
consumes ST directly with no transpose of the attention matrix. The bias add
initializes each score PSUM group via an identity matmul (start=True) — a
PSUM accumulation group whose first matmul has partial contraction rows
(tile row size < 128) faults at runtime on this toolchain, so the full-row
ident matmul must come first. The 1/sqrt(E) scale is folded into Wq once at
load time. Projections are issued before the bias stream so the PE has work
while the first xF chunks arrive.
"""

import os
from contextlib import ExitStack

import numpy as np

N = 256
D = 128
H = 4
E = 32
NCORES = 8
RPC = N // NCORES  # rows per core
SCALE = float(E) ** -0.5

_cache = {}

_legal_ctr = [0]


def _legalize_waits(nc):
    """Walrus caps semaphore wait-commands per lowered instruction (LDWEIGHTS
    holds only one). Hoist excess waits of every non-Drain instruction into
    fresh single-wait NoOps on the same engine, inserted right before it —
    same wait point, so timing/deadlock semantics are unchanged."""
    import bass_rust

    for fn in nc.m.functions:
        for blk in fn.blocks:
            ins = blk.instructions
            i = 0
            while i < len(ins):
                inst = ins[i]
                si = inst.sync_info
                if si is None or inst.engine is None:
                    i += 1
                    continue
                waits = si.on_wait
                if len(waits) <= 1:
                    i += 1
                    continue
                for w in waits[:-1]:
                    _legal_ctr[0] += 1
                    n = bass_rust.InstNoOp(name=f"I-lgl-{_legal_ctr[0]}")
                    n.engine = inst.engine
                    n.sync_info = bass_rust.SyncInfo(on_wait=[w], on_update=[])
                    ins.insert(i, n)
                    i += 1
                si.on_wait = [waits[-1]]
                inst.sync_info = si
                i += 1


def _build():
    import concourse.bass as bass
    import concourse.mybir as mybir
    import concourse.tile as tile
    from concourse.masks import make_identity

    f32 = mybir.dt.float32
    bf16 = mybir.dt.bfloat16
    AF = mybir.ActivationFunctionType
    nc = bass.Bass("TRN2", target_bir_lowering=False, debug=False,
                   enable_asserts=False, num_devices=NCORES)
    xT = nc.dram_tensor("xT", [D, RPC * N], f32, kind="ExternalInput").ap()
    xF = nc.dram_tensor("xF", [D, N * N], bf16, kind="ExternalInput").ap()
    wb = nc.dram_tensor("wb", [D, H], bf16, kind="ExternalInput").ap()
    wq = nc.dram_tensor("wq", [D, D], f32, kind="ExternalInput").ap()
    wk = nc.dram_tensor("wk", [D, D], f32, kind="ExternalInput").ap()
    wv = nc.dram_tensor("wv", [D, D], f32, kind="ExternalInput").ap()
    wo = nc.dram_tensor("wo", [D, D], f32, kind="ExternalInput").ap()
    outT = nc.dram_tensor("outT", [RPC, D, N], f32, kind="ExternalOutput").ap()

    RB = 8       # rows per projection batch
    XCH = 4096   # xF streaming chunk (columns); 16 j-blocks per chunk
    with ExitStack() as ctx:
        tc = ctx.enter_context(tile.TileContext(nc))
        singles = ctx.enter_context(tc.tile_pool(name="singles", bufs=1))
        xf_pool = ctx.enter_context(tc.tile_pool(name="xf", bufs=2))
        qk_pool = ctx.enter_context(tc.tile_pool(name="qk", bufs=4))
        v_pool = ctx.enter_context(tc.tile_pool(name="v", bufs=3))
        es_pool = ctx.enter_context(tc.tile_pool(name="es", bufs=4))
        sm_pool = ctx.enter_context(tc.tile_pool(name="sm", bufs=3))
        out_pool = ctx.enter_context(tc.tile_pool(name="outp", bufs=3))
        s_psum = ctx.enter_context(tc.tile_pool(name="spsum", bufs=2, space="PSUM"))
        o_psum = ctx.enter_context(tc.tile_pool(name="opsum", bufs=2, space="PSUM"))
        m_psum = ctx.enter_context(tc.tile_pool(name="mpsum", bufs=2, space="PSUM"))

        wb_sb = singles.tile([D, H], bf16)
        wq_sb = singles.tile([D, D], f32)
        wk_sb = singles.tile([D, D], f32)
        wv_sb = singles.tile([D, D], f32)
        wo_sb = singles.tile([D, D], f32)
        ones = singles.tile([128, E], f32)
        ident = singles.tile([128, 128], f32)
        bias_sb = singles.tile([128, 2 * H * N], f32)  # [k, kh*1024 + h*256 + j]
        xt_sb = singles.tile([D, RPC * N], f32)

        nc.sync.dma_start(out=wb_sb, in_=wb)
        nc.sync.dma_start(out=wq_sb, in_=wq)
        nc.sync.dma_start(out=wk_sb, in_=wk)
        nc.sync.dma_start(out=wv_sb, in_=wv)
        nc.sync.dma_start(out=wo_sb, in_=wo)
        nc.vector.memset(ones, 1.0)
        make_identity(nc, ident)
        nc.vector.tensor_scalar_mul(wq_sb, wq_sb, SCALE)
        for c in range(8):
            sl = slice(c * RPC * N // 8, (c + 1) * RPC * N // 8)
            nc.sync.dma_start(out=xt_sb[:, sl], in_=xT[:, sl])

        # --- projections for all rows up front (independent of xF, so the
        # PE has work while the bias stream arrives): qT/kT [he, row*N + tok]
        qTs, kTs = [], []
        for rb in range(RPC // RB):
            qT = qk_pool.tile([128, RB * N], f32, tag=f"qT{rb}", bufs=1)
            kT = qk_pool.tile([128, RB * N], f32, tag=f"kT{rb}", bufs=1)
            qTs.append(qT)
            kTs.append(kT)
            for wsb, dst in ((wq_sb, qT), (wk_sb, kT)):
                for c in range(RB * N // 512):
                    pp = m_psum.tile([128, 512], f32, tag="m")
                    nc.tensor.matmul(
                        pp,
                        wsb,
                        xt_sb[:, rb * RB * N + c * 512: rb * RB * N + (c + 1) * 512],
                        start=True, stop=True)
                    nc.vector.tensor_copy(dst[:, c * 512:(c + 1) * 512], pp)

        # --- full bias from the replicated pairwise tensor (bf16 stream).
        # bias_sb[k, kh*H*N + h*N + j] = sum_d xF[d, j*N + kh*128 + k] wb[d,h]
        bias_4d = bias_sb.rearrange("p (kh h j) -> p kh h j", kh=2, h=H)
        for c in range(N * N // XCH):
            xc = xf_pool.tile([128, XCH], bf16, tag="xf")
            nc.sync.dma_start(out=xc, in_=xF[:, c * XCH:(c + 1) * XCH])
            JB = XCH // N  # j-blocks in this chunk
            pb = m_psum.tile([128, JB * 2 * H], f32, tag="m")  # [k, (jl kh h)]
            for jl in range(JB):
                for kh in range(2):
                    nc.tensor.matmul(
                        pb[:, (jl * 2 + kh) * H:(jl * 2 + kh + 1) * H],
                        xc[:, jl * N + kh * 128: jl * N + kh * 128 + 128],
                        wb_sb,
                        start=True, stop=True,
                    )
            nc.vector.tensor_copy(
                bias_4d[:, :, :, c * JB:(c + 1) * JB],
                pb.rearrange("p (j kh h) -> p kh h j", j=JB, kh=2),
            )

        for rb in range(RPC // RB):
            qT, kT = qTs[rb], kTs[rb]
            for rl in range(RB):
                r = rb * RB + rl
                roff = rb * RB * N + rl * N
                # --- v for this row, augmented with ones for the fused
                # AV+rowsum matmul: va[k, (kh h) * 64 + (0:32 v | 32:64 ones)]
                va = v_pool.tile([128, 512], f32, tag="v")
                va_4d = va.rearrange("p (g c) -> p g c", g=8)  # g=(kh h), c=64
                pv = m_psum.tile([128, 512], f32, tag="m")
                for half in range(2):
                    nc.tensor.matmul(
                        pv[:, half * 128:(half + 1) * 128],
                        xt_sb[:, roff + half * 128: roff + half * 128 + 128],
                        wv_sb,
                        start=True, stop=True)
                # pv[k, kh*128 + h*32 + e] -> va[k, (kh*4+h)*64 + e]
                nc.vector.tensor_copy(
                    va_4d[:, :, 0:32],
                    pv[:, 0:N].rearrange("p (g e) -> p g e", g=8))
                nc.vector.memset(va_4d[:, :, 32:64], 1.0)

                # --- scores + exp, per k-half chunk [128, H*N]
                est = []
                for kh in range(2):
                    sp = s_psum.tile([128, H * N], f32, tag="s")
                    for h in range(H):
                        nc.tensor.matmul(
                            sp[:, h * N:(h + 1) * N],
                            ident,
                            bias_sb[:, kh * H * N + h * N: kh * H * N + (h + 1) * N],
                            start=True, stop=False)
                        nc.tensor.matmul(
                            sp[:, h * N:(h + 1) * N],
                            kT[32 * h:32 * h + 32, rl * N + kh * 128: rl * N + kh * 128 + 128],
                            qT[32 * h:32 * h + 32, rl * N: (rl + 1) * N],
                            start=False, stop=True,
                            tile_position=(32 * h, 0))
                    es = es_pool.tile([128, H * N], f32, tag="es")
                    nc.scalar.activation(es, sp, AF.Exp)
                    est.append(es)

                # --- fused AV + rowsum: stationary [v_h | ones] (64 cols);
                # head pairs in column halves of one PSUM tile, rows are
                # bands AVh,RSh (h even: rows 0:64, h odd: rows 64:128).
                po = o_psum.tile([128, 512], f32, tag="o")
                for h in range(H):
                    cpos = 64 * (h % 2)
                    csl = slice((h // 2) * N, (h // 2 + 1) * N)
                    for kh in range(2):
                        nc.tensor.matmul(
                            po[cpos:cpos + 64, csl],
                            va[:, (kh * 4 + h) * 64:(kh * 4 + h) * 64 + 64],
                            est[kh][:, h * N:(h + 1) * N],
                            start=(kh == 0), stop=(kh == 1),
                            tile_position=(0, cpos))

                # --- normalize: oT[he, j] = AV / RS, bands of 32 rows
                rs_rec = sm_pool.tile([128, N], f32, tag="rs")
                oT_sb = sm_pool.tile([128, N], f32, tag="oT")
                for h in range(H):
                    b = 64 * (h % 2)
                    csl = slice((h // 2) * N, (h // 2 + 1) * N)
                    nc.vector.reciprocal(
                        rs_rec[32 * h:32 * h + 32, :], po[b + 32:b + 64, csl])
                    nc.vector.tensor_mul(
                        oT_sb[32 * h:32 * h + 32, :],
                        po[b:b + 32, csl],
                        rs_rec[32 * h:32 * h + 32, :])

                # --- output projection: outT[d, j] = sum_he Wo[he,d] oT[he,j]
                pf = m_psum.tile([128, 512], f32, tag="m")
                nc.tensor.matmul(pf[:, 0:N], wo_sb, oT_sb, start=True, stop=True)
                o_sb = out_pool.tile([128, N], f32, tag="osb")
                nc.vector.tensor_copy(o_sb, pf[:, 0:N])
                nc.sync.dma_start(out=outT[r], in_=o_sb)
    return nc


def _get_program():
    if "nc" not in _cache:
        _cache["nc"] = _build()
        _legalize_waits(_cache["nc"])
    return _cache["nc"]


def kernel(pairwise_repr, mask, Wb, Wq, Wk, Wv, Wo):
    import ml_dtypes
    from concourse.bass_utils import run_bass_kernel_spmd

    nc = _get_program()

    x = np.ascontiguousarray(np.asarray(pairwise_repr, dtype=np.float32)[0])
    # xT[d, i*N + t] = x[i, t, d]
    xT = np.ascontiguousarray(x.reshape(N * N, D).T)
    xF = xT.astype(ml_dtypes.bfloat16)
    shards = [np.ascontiguousarray(xT[:, c * RPC * N:(c + 1) * RPC * N])
              for c in range(NCORES)]
    wb = np.asarray(Wb, np.float32).astype(ml_dtypes.bfloat16)
    wq = np.ascontiguousarray(np.asarray(Wq, np.float32))
    wk = np.ascontiguousarray(np.asarray(Wk, np.float32))
    wv = np.ascontiguousarray(np.asarray(Wv, np.float32))
    wo = np.ascontiguousarray(np.asarray(Wo, np.float32))

    core_ids = list(range(NCORES))
    in_maps = [{"xT": shards[c], "xF": xF, "wb": wb, "wq": wq, "wk": wk,
                "wv": wv, "wo": wo} for c in range(NCORES)]
    kernel._last_in = in_maps
    res = run_bass_kernel_spmd(nc, in_maps, core_ids=core_ids, trace=False)

    kernel._last = res
    # outT [RPC, D, N] per core -> out[0, 32c+r, j, d] = outT_c[r, d, j]
    o = np.stack([res.results[c]["outT"] for c in range(NCORES)])
    out = o.transpose(0, 1, 3, 2).reshape(1, N, N, D)
    return np.ascontiguousarray(out.astype(np.float32))


# revision 16
# speedup vs baseline: 91.7765x; 1.0954x over previous
"""Triangle attention (starting node) Bass kernel for 8 trn2 NeuronCores.

Math (B=1, N=256, D=128, H=4, E=32):
  bias[h,j,k] = sum_d P[j,k,d] Wb[d,h]
  q[h,i,j,e]  = sum_d P[i,j,d] Wq[d,h*E+e]   (k,v analogous)
  S[i,h,j,k]  = (q . k) * E**-0.5 + bias[h,j,k]
  out[i,j,:]  = (softmax_k S @ v) merged over h, @ Wo

Sharding: rows i are split across 8 cores (32 rows each). The bias couples all
rows, so each core receives the FULL pairwise tensor (replicated input, in
transposed [d, j*N+k] form, bf16 to halve the stream) and recomputes the full
bias on-chip — one kernel launch, no host round trip between a bias pass and
an attention pass. All PE operands are bf16 (fp32 matmuls run at 1/4 moving
rate on TRN2; bf16 runs at full rate) with fp32 PSUM accumulation — well
within the correctness budget.

On-chip layout is "T-form": scores are built transposed, ST[k, j] per head, so
softmax normalization sums over the partition axis (done on the PE together
with the AV matmul: the stationary operand is [v_h | ones], so one pass yields
both A@V and the softmax denominator replicated x32) and the AV matmul
consumes ST directly with no transpose of the attention matrix. The bias add
initializes each score PSUM group via an identity matmul (start=True) — a
PSUM accumulation group whose first matmul has partial contraction rows
(tile row size < 128) faults at runtime on this toolchain, so the full-row
ident matmul must come first. The 1/sqrt(E) scale is folded into Wq once at
load time. Projections are issued before the bias stream so the PE has work
while the first xF chunks arrive.
"""

import os
from contextlib import ExitStack

import numpy as np

N = 256
D = 128
H = 4
E = 32
NCORES = 8
RPC = N // NCORES  # rows per core
SCALE = float(E) ** -0.5

_cache = {}

_legal_ctr = [0]


def _legalize_waits(nc):
    """Walrus caps semaphore wait-commands per lowered instruction (LDWEIGHTS
    holds only one). Hoist excess waits of every non-Drain instruction into
    fresh single-wait NoOps on the same engine, inserted right before it —
    same wait point, so timing/deadlock semantics are unchanged."""
    import bass_rust

    for fn in nc.m.functions:
        for blk in fn.blocks:
            ins = blk.instructions
            i = 0
            while i < len(ins):
                inst = ins[i]
                si = inst.sync_info
                if si is None or inst.engine is None:
                    i += 1
                    continue
                waits = si.on_wait
                if len(waits) <= 1:
                    i += 1
                    continue
                for w in waits[:-1]:
                    _legal_ctr[0] += 1
                    n = bass_rust.InstNoOp(name=f"I-lgl-{_legal_ctr[0]}")
                    n.engine = inst.engine
                    n.sync_info = bass_rust.SyncInfo(on_wait=[w], on_update=[])
                    ins.insert(i, n)
                    i += 1
                si.on_wait = [waits[-1]]
                inst.sync_info = si
                i += 1


def _build():
    import concourse.bass as bass
    import concourse.mybir as mybir
    import concourse.tile as tile
    from concourse.masks import make_identity

    f32 = mybir.dt.float32
    bf16 = mybir.dt.bfloat16
    AF = mybir.ActivationFunctionType
    nc = bass.Bass("TRN2", target_bir_lowering=False, debug=False,
                   enable_asserts=False, num_devices=NCORES)
    xT = nc.dram_tensor("xT", [D, RPC * N], bf16, kind="ExternalInput").ap()
    xF = nc.dram_tensor("xF", [D, N * N], bf16, kind="ExternalInput").ap()
    wb = nc.dram_tensor("wb", [D, H], bf16, kind="ExternalInput").ap()
    wq = nc.dram_tensor("wq", [D, D], bf16, kind="ExternalInput").ap()
    wk = nc.dram_tensor("wk", [D, D], bf16, kind="ExternalInput").ap()
    wv = nc.dram_tensor("wv", [D, D], bf16, kind="ExternalInput").ap()
    wo = nc.dram_tensor("wo", [D, D], bf16, kind="ExternalInput").ap()
    outT = nc.dram_tensor("outT", [RPC, D, N], f32, kind="ExternalOutput").ap()

    RB = 8       # rows per projection batch
    XCH = 4096   # xF streaming chunk (columns); 16 j-blocks per chunk
    with ExitStack() as ctx:
        tc = ctx.enter_context(tile.TileContext(nc))
        singles = ctx.enter_context(tc.tile_pool(name="singles", bufs=1))
        xf_pool = ctx.enter_context(tc.tile_pool(name="xf", bufs=4))
        qk_pool = ctx.enter_context(tc.tile_pool(name="qk", bufs=4))
        v_pool = ctx.enter_context(tc.tile_pool(name="v", bufs=3))
        es_pool = ctx.enter_context(tc.tile_pool(name="es", bufs=4))
        sm_pool = ctx.enter_context(tc.tile_pool(name="sm", bufs=3))
        out_pool = ctx.enter_context(tc.tile_pool(name="outp", bufs=3))
        s_psum = ctx.enter_context(tc.tile_pool(name="spsum", bufs=2, space="PSUM"))
        o_psum = ctx.enter_context(tc.tile_pool(name="opsum", bufs=2, space="PSUM"))
        m_psum = ctx.enter_context(tc.tile_pool(name="mpsum", bufs=2, space="PSUM"))

        wb_sb = singles.tile([D, H], bf16)
        wq_sb = singles.tile([D, D], bf16)
        wk_sb = singles.tile([D, D], bf16)
        wv_sb = singles.tile([D, D], bf16)
        wo_sb = singles.tile([D, D], bf16)
        ident = singles.tile([128, 128], bf16)
        bias_sb = singles.tile([128, 2 * H * N], bf16)  # [k, kh*1024 + h*256 + j]
        xt_sb = singles.tile([D, RPC * N], bf16)

        nc.sync.dma_start(out=wb_sb, in_=wb)
        nc.sync.dma_start(out=wq_sb, in_=wq)
        nc.sync.dma_start(out=wk_sb, in_=wk)
        nc.sync.dma_start(out=wv_sb, in_=wv)
        nc.sync.dma_start(out=wo_sb, in_=wo)
        make_identity(nc, ident)
        nc.vector.tensor_scalar_mul(wq_sb, wq_sb, SCALE)
        for c in range(8):
            sl = slice(c * RPC * N // 8, (c + 1) * RPC * N // 8)
            nc.sync.dma_start(out=xt_sb[:, sl], in_=xT[:, sl])

        # --- projections for all rows up front (independent of xF, so the
        # PE has work while the bias stream arrives): qT/kT [he, row*N + tok]
        qTs, kTs = [], []
        for rb in range(RPC // RB):
            qT = qk_pool.tile([128, RB * N], bf16, tag=f"qT{rb}", bufs=1)
            kT = qk_pool.tile([128, RB * N], bf16, tag=f"kT{rb}", bufs=1)
            qTs.append(qT)
            kTs.append(kT)
            for wsb, dst in ((wq_sb, qT), (wk_sb, kT)):
                for c in range(RB * N // 512):
                    pp = m_psum.tile([128, 512], f32, tag="m")
                    nc.tensor.matmul(
                        pp,
                        wsb,
                        xt_sb[:, rb * RB * N + c * 512: rb * RB * N + (c + 1) * 512],
                        start=True, stop=True)
                    nc.vector.tensor_copy(dst[:, c * 512:(c + 1) * 512], pp)

        # --- full bias from the replicated pairwise tensor (bf16 stream).
        # bias_sb[k, kh*H*N + h*N + j] = sum_d xF[d, j*N + kh*128 + k] wb[d,h]
        bias_4d = bias_sb.rearrange("p (kh h j) -> p kh h j", kh=2, h=H)
        for c in range(N * N // XCH):
            xc = xf_pool.tile([128, XCH], bf16, tag="xf")
            nc.sync.dma_start(out=xc, in_=xF[:, c * XCH:(c + 1) * XCH])
            JB = XCH // N  # j-blocks in this chunk
            pb = m_psum.tile([128, JB * 2 * H], f32, tag="m")  # [k, (jl kh h)]
            for jl in range(JB):
                for kh in range(2):
                    nc.tensor.matmul(
                        pb[:, (jl * 2 + kh) * H:(jl * 2 + kh + 1) * H],
                        xc[:, jl * N + kh * 128: jl * N + kh * 128 + 128],
                        wb_sb,
                        start=True, stop=True,
                    )
            nc.vector.tensor_copy(
                bias_4d[:, :, :, c * JB:(c + 1) * JB],
                pb.rearrange("p (j kh h) -> p kh h j", j=JB, kh=2),
            )

        for rb in range(RPC // RB):
            qT, kT = qTs[rb], kTs[rb]
            for rl in range(RB):
                r = rb * RB + rl
                roff = rb * RB * N + rl * N
                # --- v for this row, augmented with ones for the fused
                # AV+rowsum matmul: va[k, (kh h) * 64 + (0:32 v | 32:64 ones)]
                va = v_pool.tile([128, 512], bf16, tag="v")
                va_4d = va.rearrange("p (g c) -> p g c", g=8)  # g=(kh h), c=64
                pv = m_psum.tile([128, 512], f32, tag="m")
                for half in range(2):
                    nc.tensor.matmul(
                        pv[:, half * 128:(half + 1) * 128],
                        xt_sb[:, roff + half * 128: roff + half * 128 + 128],
                        wv_sb,
                        start=True, stop=True)
                # pv[k, kh*128 + h*32 + e] -> va[k, (kh*4+h)*64 + e]
                nc.vector.tensor_copy(
                    va_4d[:, :, 0:32],
                    pv[:, 0:N].rearrange("p (g e) -> p g e", g=8))
                nc.vector.memset(va_4d[:, :, 32:64], 1.0)

                # --- scores + exp, per k-half chunk [128, H*N]
                est = []
                for kh in range(2):
                    sp = s_psum.tile([128, H * N], f32, tag="s")
                    for h in range(H):
                        nc.tensor.matmul(
                            sp[:, h * N:(h + 1) * N],
                            ident,
                            bias_sb[:, kh * H * N + h * N: kh * H * N + (h + 1) * N],
                            start=True, stop=False)
                        nc.tensor.matmul(
                            sp[:, h * N:(h + 1) * N],
                            kT[32 * h:32 * h + 32, rl * N + kh * 128: rl * N + kh * 128 + 128],
                            qT[32 * h:32 * h + 32, rl * N: (rl + 1) * N],
                            start=False, stop=True,
                            tile_position=(32 * h, 0))
                    es = es_pool.tile([128, H * N], bf16, tag="es")
                    nc.scalar.activation(es, sp, AF.Exp)
                    est.append(es)

                # --- fused AV + rowsum: stationary [v_h | ones] (64 cols);
                # head pairs in column halves of one PSUM tile, rows are
                # bands AVh,RSh (h even: rows 0:64, h odd: rows 64:128).
                po = o_psum.tile([128, 512], f32, tag="o")
                for h in range(H):
                    cpos = 64 * (h % 2)
                    csl = slice((h // 2) * N, (h // 2 + 1) * N)
                    for kh in range(2):
                        nc.tensor.matmul(
                            po[cpos:cpos + 64, csl],
                            va[:, (kh * 4 + h) * 64:(kh * 4 + h) * 64 + 64],
                            est[kh][:, h * N:(h + 1) * N],
                            start=(kh == 0), stop=(kh == 1),
                            tile_position=(0, cpos))

                # --- normalize: oT[he, j] = AV / RS, bands of 32 rows
                rs_rec = sm_pool.tile([128, N], f32, tag="rs")
                oT_sb = sm_pool.tile([128, N], bf16, tag="oT")
                for h in range(H):
                    b = 64 * (h % 2)
                    csl = slice((h // 2) * N, (h // 2 + 1) * N)
                    nc.vector.reciprocal(
                        rs_rec[32 * h:32 * h + 32, :], po[b + 32:b + 64, csl])
                    nc.vector.tensor_mul(
                        oT_sb[32 * h:32 * h + 32, :],
                        po[b:b + 32, csl],
                        rs_rec[32 * h:32 * h + 32, :])

                # --- output projection: outT[d, j] = sum_he Wo[he,d] oT[he,j]
                pf = m_psum.tile([128, 512], f32, tag="m")
                nc.tensor.matmul(pf[:, 0:N], wo_sb, oT_sb, start=True, stop=True)
                o_sb = out_pool.tile([128, N], f32, tag="osb")
                nc.vector.tensor_copy(o_sb, pf[:, 0:N])
                nc.sync.dma_start(out=outT[r], in_=o_sb)
    return nc


def _get_program():
    if "nc" not in _cache:
        _cache["nc"] = _build()
        _legalize_waits(_cache["nc"])
    return _cache["nc"]


def kernel(pairwise_repr, mask, Wb, Wq, Wk, Wv, Wo):
    import ml_dtypes
    from concourse.bass_utils import run_bass_kernel_spmd

    nc = _get_program()

    x = np.ascontiguousarray(np.asarray(pairwise_repr, dtype=np.float32)[0])
    # xT[d, i*N + t] = x[i, t, d]
    xT = np.ascontiguousarray(x.reshape(N * N, D).T)
    xF = xT.astype(ml_dtypes.bfloat16)
    shards = [np.ascontiguousarray(xF[:, c * RPC * N:(c + 1) * RPC * N])
              for c in range(NCORES)]
    wb = np.asarray(Wb, np.float32).astype(ml_dtypes.bfloat16)
    wq = np.asarray(Wq, np.float32).astype(ml_dtypes.bfloat16)
    wk = np.asarray(Wk, np.float32).astype(ml_dtypes.bfloat16)
    wv = np.asarray(Wv, np.float32).astype(ml_dtypes.bfloat16)
    wo = np.asarray(Wo, np.float32).astype(ml_dtypes.bfloat16)

    core_ids = list(range(NCORES))
    in_maps = [{"xT": shards[c], "xF": xF, "wb": wb, "wq": wq, "wk": wk,
                "wv": wv, "wo": wo} for c in range(NCORES)]
    kernel._last_in = in_maps
    res = run_bass_kernel_spmd(nc, in_maps, core_ids=core_ids, trace=False)

    kernel._last = res
    # outT [RPC, D, N] per core -> out[0, 32c+r, j, d] = outT_c[r, d, j]
    o = np.stack([res.results[c]["outT"] for c in range(NCORES)])
    out = o.transpose(0, 1, 3, 2).reshape(1, N, N, D)
    return np.ascontiguousarray(out.astype(np.float32))
